# revision 23
# baseline (speedup 1.0000x reference)
"""DifferentialDropout TRN2 kernel.

Strategy (8 NeuronCores, column sharding):
  Each core owns a contiguous 8192-column slice of x/noise (full 256 rows).
  Phase A (local, per core):
    - G_k = X_k @ X_k^T  (256x256 partial Gram over the local columns) on PE,
      via per-128-column PE transpose + f32r matmuls accumulating in PSUM.
    - mean_k[i] = avg of x[i, local cols]  (Pool engine avg-pool)
    - hist_k[i] = OR over local cols of (1 << (round(x)+16))  (int32 bitmask
      of occupied integer bins; round == RNE via the fp32 magic-number trick,
      exactly matching jnp.round)
  AllGather of the 256x258 payload [G | mean | hist] (DRAM, 264KB).
  Phase B (every core, redundant 256-sized math):
    full G = sum_k G_k;  s8[i] = sum_k mean_k[i] (= 8*mean_i)
    mse*F = diag(G) - (2/B) rowsum(G) + totsum(G)/B^2   (scale-invariant in
      factor2 = mse_i / sum(mse))
    C = G - 1024 * s8_i * s8_j  (centered Gram);  d = 1/sqrt(diag C)
    factor1_i = (1/B) d_i * sum_j |C_ij| d_j
    row_unique = popcount(OR_k hist_k[i]);  total_unique = popcount(OR_all)
    p_i = 1 - (f1 + (1-f2) + (1-f3)) / (3*factor4)
  Phase C (local): out = (noise > p_i) * x * (1/(1-p[255])), streamed out.

All numerics fp32; Gram in float32r (full-rate fp32 matmul).
"""

import numpy as np
from contextlib import ExitStack

import concourse.bass as bass
import concourse.tile as tile
from concourse import mybir
from concourse.masks import make_identity
from concourse.bass_isa import ReduceOp

B = 256          # rows
F = 65536        # total columns
NCORES = 8
FS = F // NCORES  # 8192 columns per core
NB = 2            # row blocks of 128
P = 128
HCHUNK = 1024     # hist pass chunk width
OCHUNK = 512      # output chunk width
MAGIC = 12582912.0  # 1.5 * 2**23 : fp32 RNE-to-integer magic
GRAM_F32R = True  # f32r matmul: 4x faster PE, reduced mantissa

f32 = mybir.dt.float32
f32r = mybir.dt.float32r
i32 = mybir.dt.int32
Alu = mybir.AluOpType
AX = mybir.AxisListType


def build_program(inv3f4: float):
    """Build the 8-core SPMD Bass program. inv3f4 = float32(1/(3*factor4))."""
    import concourse.bacc as bacc
    NCHUNK = FS // P          # transpose/matmul chunks
    SCOEF = 1.0 / F           # s_i*s_j scale for centering (power of 2)
    PAYW = B + 2              # payload cols: G | mean | hist
    nc = bacc.Bacc("TRN2", target_bir_lowering=False, debug=False,
                   num_devices=NCORES)
    x_in = nc.dram_tensor("x", [B, FS], f32, kind="ExternalInput")
    n_in = nc.dram_tensor("noise", [B, FS], f32, kind="ExternalInput")
    out_d = nc.dram_tensor("out", [B, FS], f32, kind="ExternalOutput")

    with tile.TileContext(nc) as tc, ExitStack() as ctx:
        big = ctx.enter_context(tc.tile_pool(name="big", bufs=1))
        xt = big.tile([P, NB, FS], f32)    # 64KB/part

        singles = ctx.enter_context(tc.tile_pool(name="singles", bufs=1))
        tpool = ctx.enter_context(tc.tile_pool(name="tp", bufs=3))
        kpool = ctx.enter_context(tc.tile_pool(name="kp", bufs=2))
        mpool = ctx.enter_context(tc.tile_pool(name="mp", bufs=2))
        m2pool = ctx.enter_context(tc.tile_pool(name="mp2", bufs=2))
        fold = ctx.enter_context(tc.tile_pool(name="fold", bufs=2))
        opool = ctx.enter_context(tc.tile_pool(name="op", bufs=4))
        small = ctx.enter_context(tc.tile_pool(name="small", bufs=8))
        onep = ctx.enter_context(tc.tile_pool(name="onep", bufs=1))
        psg = ctx.enter_context(tc.tile_pool(name="psg", bufs=1, space="PSUM"))
        pst = ctx.enter_context(tc.tile_pool(name="pst", bufs=3, space="PSUM"))
        dram = ctx.enter_context(tc.tile_pool(name="dram", bufs=1, space="DRAM"))

        # ---- input DMA (x first; noise later, overlaps gram/collective) ----
        NXD = 8
        xw = FS // NXD
        for c in range(NXD):
            sl = slice(c * xw, (c + 1) * xw)
            nc.sync.dma_start(
                out=xt[:, :, sl],
                in_=x_in.ap()[:, sl].rearrange("(b p) f -> p b f", p=P))

        # ---- constants ----
        ident = singles.tile([P, P], f32)
        make_identity(nc, ident)
        one_i = singles.tile([P, 1], i32)
        nc.vector.memset(one_i, 1)

        # ---- Gram + row sums: rhs = [Xt_c | ones], psum col 256 = rowsums ----
        gdt = f32r if GRAM_F32R else f32
        c10 = singles.tile([P, 2], f32)     # [1.0, 0.0]: ones col + even-pad col
        nc.vector.memset(c10[:, 0:1], 1.0)
        nc.vector.memset(c10[:, 1:2], 0.0)
        xtcc = [singles.tile([P, B + 2], gdt, name=f"xtcc{i}") for i in range(3)]
        for i in range(3):
            nc.scalar.copy(out=xtcc[i][:, B:B + 2], in_=c10)
        gps = [psg.tile([P, B + 2], f32, name=f"gps{b}") for b in range(NB)]
        for c in range(NCHUNK):
            fsl = slice(c * P, (c + 1) * P)
            tp = pst.tile([P, B], f32)
            nc.tensor.transpose(tp[:, 0:P], xt[:, 0, fsl], ident)
            nc.tensor.transpose(tp[:, P:B], xt[:, 1, fsl], ident)
            xtc = xtcc[c % 3]
            nc.scalar.copy(out=xtc[:, 0:B], in_=tp)
            for b in range(NB):
                nc.tensor.matmul(gps[b],
                                 lhsT=xtc[:, b * P:(b + 1) * P],
                                 rhs=xtc,
                                 start=(c == 0), stop=(c == NCHUNK - 1))

        # ---- payload [G | rowsum | hist] ----
        payload = singles.tile([P, NB, PAYW], f32)
        for b in range(NB):
            nc.scalar.copy(out=payload[:, b, 0:B + 1], in_=gps[b][:, 0:B + 1])

        # hist bitmask, shift-free: k = RNE-round(x)+143 (fused magic round),
        # e = k * 2^23 (int mult on Pool) == f32 bits of 2.0^(round(x)+16),
        # m = int32(bitcast_f32(e)) == 1 << (round(x)+16), then OR-fold.
        c8m = singles.tile([P, 1], i32)
        nc.vector.memset(c8m, 1 << 23)
        hist = singles.tile([P, NB], i32)
        nc.vector.memset(hist, 0)
        for b in range(NB):
            for hc in range(FS // HCHUNK):
                sl = slice(hc * HCHUNK, (hc + 1) * HCHUNK)
                k = kpool.tile([P, HCHUNK], i32)
                nc.vector.tensor_scalar(out=k, in0=xt[:, b, sl],
                                        scalar1=MAGIC, scalar2=MAGIC - 143.0,
                                        op0=Alu.add, op1=Alu.subtract)
                e = mpool.tile([P, HCHUNK], i32)
                nc.gpsimd.tensor_tensor(out=e, in0=k,
                                        in1=c8m.to_broadcast([P, HCHUNK]),
                                        op=Alu.mult)
                m = m2pool.tile([P, HCHUNK], i32)
                nc.scalar.copy(out=m, in_=e.bitcast(f32))
                w = HCHUNK
                src = m
                while w > 1:
                    w //= 2
                    nxt = fold.tile([P, w], i32)
                    nc.vector.tensor_tensor(out=nxt, in0=src[:, 0:w],
                                            in1=src[:, w:2 * w],
                                            op=Alu.bitwise_or)
                    src = nxt
                nc.vector.tensor_tensor(out=hist[:, b:b + 1], in0=hist[:, b:b + 1],
                                        in1=src, op=Alu.bitwise_or)
        nc.vector.tensor_copy(out=payload[:, :, PAYW - 1:PAYW].bitcast(i32),
                              in_=hist.rearrange("p (b o) -> p b o", o=1))

        # ---- AllGather payload ----
        pay_d = dram.tile([P, NB, PAYW], f32)
        gath_d = dram.tile([NCORES, P, NB, PAYW], f32)
        nc.sync.dma_start(out=pay_d, in_=payload)
        nc.gpsimd.collective_compute(
            "AllGather", Alu.bypass,
            replica_groups=[list(range(NCORES))],
            ins=[pay_d[:]], outs=[gath_d[:]])

        gsb = singles.tile([P, NCORES, NB, PAYW], f32)  # 16.5KB/part
        nc.sync.dma_start(out=gsb, in_=gath_d.rearrange("n p b c -> p n b c"))

        # ---- combine cores ----
        acc = singles.tile([P, NB, B + 1], f32)  # G (256) + s8 (1)
        nc.vector.tensor_reduce(
            out=acc, in_=gsb.rearrange("p n b c -> p b c n")[:, :, 0:B + 1, :],
            axis=AX.X, op=Alu.add)
        G = acc[:, :, 0:B]
        s8 = acc[:, :, B:B + 1]            # [P, NB, 1] = full row sum

        histg = gsb[:, :, :, PAYW - 1:PAYW].bitcast(i32)  # [P, 8, NB, 1]
        h4 = small.tile([P, 4, NB], i32)
        nc.vector.tensor_tensor(out=h4, in0=histg[:, 0:4, :, 0],
                                in1=histg[:, 4:8, :, 0], op=Alu.bitwise_or)
        h2 = small.tile([P, 2, NB], i32)
        nc.vector.tensor_tensor(out=h2, in0=h4[:, 0:2, :], in1=h4[:, 2:4, :],
                                op=Alu.bitwise_or)
        histf = small.tile([P, NB], i32)   # full-row bin mask
        nc.vector.tensor_tensor(out=histf, in0=h2[:, 0, :], in1=h2[:, 1, :],
                                op=Alu.bitwise_or)

        # ---- diag/rowsum of G ----
        eye2 = singles.tile([P, NB, B], f32)
        nc.vector.memset(eye2, 0.0)
        for b in range(NB):
            nc.vector.tensor_copy(out=eye2[:, b, b * P:(b + 1) * P], in_=ident)
        gd = small.tile([P, NB, B], f32)
        nc.vector.tensor_tensor(out=gd, in0=G, in1=eye2, op=Alu.mult)
        diagG = small.tile([P, NB], f32)
        nc.vector.tensor_reduce(out=diagG, in_=gd, axis=AX.X, op=Alu.add)
        rowsG = small.tile([P, NB], f32)
        nc.vector.tensor_reduce(out=rowsG, in_=G, axis=AX.X, op=Alu.add)
        # partition reductions via DMA round trip: pack [rowsum-total, diag-total]
        pr2 = small.tile([P, 2], f32)
        nc.vector.tensor_reduce(out=pr2[:, 0:1], in_=rowsG, axis=AX.X, op=Alu.add)
        nc.vector.tensor_reduce(out=pr2[:, 1:2], in_=diagG, axis=AX.X, op=Alu.add)
        pr_d = dram.tile([P, 2], f32)
        nc.sync.dma_start(out=pr_d, in_=pr2)
        prt = onep.tile([1, 2, P], f32)
        nc.sync.dma_start(out=prt,
                          in_=bass.AP(tensor=pr_d.tensor, offset=pr_d.offset,
                                      ap=[[0, 1], [1, 2], [2, P]]))
        pr1 = small.tile([1, 2], f32)     # [totG, sum diagG]
        nc.vector.tensor_reduce(out=pr1, in_=prt, axis=AX.X, op=Alu.add)

        # ---- centered Gram + factor1 ----
        # s8 broadcast to free dim via DRAM round trip
        srow_d = dram.tile([B], f32)
        nc.sync.dma_start(out=srow_d.rearrange("(b p) -> p b", p=P), in_=s8[:, :, 0])
        s8b = singles.tile([P, B], f32)
        nc.sync.dma_start(out=s8b, in_=bass.AP(tensor=srow_d.tensor,
                                               offset=srow_d.offset,
                                               ap=[[0, P]] + srow_d.ap))
        nB8 = small.tile([P, NB], f32)
        nc.vector.tensor_scalar(out=nB8, in0=s8, scalar1=-SCOEF, scalar2=None,
                                op0=Alu.mult)
        C = singles.tile([P, NB, B], f32)
        for b in range(NB):
            nc.vector.scalar_tensor_tensor(out=C[:, b, :], in0=s8b,
                                           scalar=nB8[:, b:b + 1],
                                           in1=G[:, b, :],
                                           op0=Alu.mult, op1=Alu.add)
        cd = small.tile([P, NB], f32)
        nc.vector.tensor_tensor(out=cd, in0=s8, in1=s8, op=Alu.mult)
        diagC = small.tile([P, NB], f32)
        nc.vector.scalar_tensor_tensor(out=diagC, in0=cd, scalar=-SCOEF,
                                       in1=diagG, op0=Alu.mult, op1=Alu.add)
        sdC = small.tile([P, NB], f32)
        nc.scalar.sqrt(out=sdC, in_=diagC)
        d = small.tile([P, NB], f32)
        nc.vector.reciprocal(out=d, in_=sdC)
        drow_d = dram.tile([B], f32)
        nc.sync.dma_start(out=drow_d.rearrange("(b p) -> p b", p=P), in_=d)
        db = singles.tile([P, B], f32)
        nc.sync.dma_start(out=db, in_=bass.AP(tensor=drow_d.tensor,
                                              offset=drow_d.offset,
                                              ap=[[0, P]] + drow_d.ap))
        v = small.tile([P, NB], f32)
        for b in range(NB):
            u = small.tile([P, B], f32)
            nc.vector.tensor_tensor(out=u, in0=C[:, b, :], in1=db, op=Alu.mult)
            nc.vector.tensor_reduce(out=v[:, b:b + 1], in_=u, axis=AX.X,
                                    op=Alu.add, apply_absolute_value=True)
        f1a = small.tile([P, NB], f32)
        nc.vector.tensor_tensor(out=f1a, in0=v, in1=d, op=Alu.mult)
        f1 = small.tile([P, NB], f32)
        nc.vector.tensor_scalar(out=f1, in0=f1a, scalar1=1.0 / B, scalar2=None,
                                op0=Alu.mult)

        # ---- unique counts ----
        def popcount(eng, t, w):
            """SWAR popcount of int32 tile t [P or 1, w] -> f32 tile."""
            def const(val):
                ct = small.tile([t.shape[0], 1], i32)
                nc.vector.memset(ct, val)
                return ct.to_broadcast([t.shape[0], w])
            def shr(src, amt):
                o = small.tile([t.shape[0], w], i32)
                eng.tensor_tensor(out=o, in0=src, in1=const(amt),
                                  op=Alu.logical_shift_right)
                return o
            def band(a, bmask):
                o = small.tile([t.shape[0], w], i32)
                eng.tensor_tensor(out=o, in0=a, in1=const(bmask), op=Alu.bitwise_and)
                return o
            def add(a, b2):
                o = small.tile([t.shape[0], w], i32)
                eng.tensor_tensor(out=o, in0=a, in1=b2, op=Alu.add)
                return o
            def sub(a, b2):
                o = small.tile([t.shape[0], w], i32)
                eng.tensor_tensor(out=o, in0=a, in1=b2, op=Alu.subtract)
                return o
            v1 = sub(t, band(shr(t, 1), 0x55555555))
            v2 = add(band(v1, 0x33333333), band(shr(v1, 2), 0x33333333))
            v3 = band(add(v2, shr(v2, 4)), 0x0F0F0F0F)
            v4 = add(v3, shr(v3, 8))
            v5 = band(add(v4, shr(v4, 16)), 0x3F)
            o = small.tile([t.shape[0], w], f32)
            eng.tensor_copy(out=o, in_=v5)
            return o

        ru = popcount(nc.vector, histf, NB)   # row unique counts, f32

        # total unique: OR the 2048 per-core-block masks from gathered DRAM
        thist = onep.tile([1, NCORES * NB * P], i32)
        nc.sync.dma_start(
            out=thist,
            in_=bass.AP(tensor=gath_d.tensor, offset=gath_d.offset + (PAYW - 1),
                        ap=[[0, 1], [PAYW * NB * P, NCORES], [PAYW * NB, P],
                            [PAYW, NB]]).bitcast(i32))
        w = NCORES * NB * P
        src = thist
        while w > 1:
            w //= 2
            nxt = fold.tile([1, w], i32)
            nc.vector.tensor_tensor(out=nxt, in0=src[:, 0:w], in1=src[:, w:2 * w],
                                    op=Alu.bitwise_or)
            src = nxt
        tu = popcount(nc.vector, src, 1)      # [1,1] f32

        # broadcast [totG, sum diagG, total_unique] to all partitions via DRAM
        b3 = small.tile([1, 3], f32)
        nc.vector.tensor_copy(out=b3[:, 0:2], in_=pr1)
        nc.vector.tensor_copy(out=b3[:, 2:3], in_=tu)
        b3_d = dram.tile([3], f32)
        nc.sync.dma_start(out=b3_d, in_=b3)
        bb = small.tile([P, 3], f32)
        nc.sync.dma_start(out=bb, in_=bass.AP(tensor=b3_d.tensor,
                                              offset=b3_d.offset,
                                              ap=[[0, P], [1, 3]]))
        totGb = bb[:, 0:1]

        # mse (x F; scale cancels in factor2). total = sum diagG - totG/B
        mse0 = small.tile([P, NB], f32)
        nc.vector.scalar_tensor_tensor(out=mse0, in0=rowsG, scalar=-2.0 / B,
                                       in1=diagG, op0=Alu.mult, op1=Alu.add)
        mse = small.tile([P, NB], f32)     # = F * mse_rows
        nc.vector.scalar_tensor_tensor(out=mse, in0=totGb.to_broadcast([P, NB]),
                                       scalar=1.0 / (B * B), in1=mse0,
                                       op0=Alu.mult, op1=Alu.add)
        tmseb = small.tile([P, 1], f32)
        nc.vector.scalar_tensor_tensor(out=tmseb, in0=bb[:, 0:1],
                                       scalar=-1.0 / B, in1=bb[:, 1:2],
                                       op0=Alu.mult, op1=Alu.add)
        rtmse = small.tile([P, 1], f32)
        nc.vector.reciprocal(out=rtmse, in_=tmseb)
        f2 = small.tile([P, NB], f32)
        nc.vector.tensor_scalar(out=f2, in0=mse, scalar1=rtmse, scalar2=None,
                                op0=Alu.mult)

        rtu = small.tile([P, 1], f32)
        nc.vector.reciprocal(out=rtu, in_=bb[:, 2:3])
        f3 = small.tile([P, NB], f32)
        nc.vector.tensor_scalar(out=f3, in0=ru, scalar1=rtu, scalar2=None,
                                op0=Alu.mult)

        # ---- p = 1 - (f1 + (1-f2) + (1-f3)) * inv3f4 ----
        t1 = small.tile([P, NB], f32)
        nc.vector.tensor_scalar(out=t1, in0=f2, scalar1=-1.0, scalar2=1.0,
                                op0=Alu.mult, op1=Alu.add)
        t2 = small.tile([P, NB], f32)
        nc.vector.tensor_tensor(out=t2, in0=f1, in1=t1, op=Alu.add)
        t3 = small.tile([P, NB], f32)
        nc.vector.tensor_scalar(out=t3, in0=f3, scalar1=-1.0, scalar2=1.0,
                                op0=Alu.mult, op1=Alu.add)
        t4 = small.tile([P, NB], f32)
        nc.vector.tensor_tensor(out=t4, in0=t2, in1=t3, op=Alu.add)
        p = small.tile([P, NB], f32)
        nc.vector.tensor_scalar(out=p, in0=t4, scalar1=-inv3f4, scalar2=1.0,
                                op0=Alu.mult, op1=Alu.add)

        # 1/(1 - p_last), p_last = p[row 255] = p[part 127, block 1]
        pl_d = dram.tile([1], f32)
        nc.sync.dma_start(out=pl_d, in_=p[127:128, 1:2])
        plastb = small.tile([P, 1], f32)
        nc.sync.dma_start(out=plastb, in_=bass.AP(tensor=pl_d.tensor,
                                                  offset=pl_d.offset,
                                                  ap=[[0, P], [1, 1]]))
        om = small.tile([P, 1], f32)
        nc.vector.tensor_scalar(out=om, in0=plastb, scalar1=-1.0, scalar2=1.0,
                                op0=Alu.mult, op1=Alu.add)
        inv = small.tile([P, 1], f32)
        nc.vector.reciprocal(out=inv, in_=om)

        # ---- masked scaled output: stream noise in, out back ----
        for oc in range(FS // OCHUNK):
            sl = slice(oc * OCHUNK, (oc + 1) * OCHUNK)
            nct = opool.tile([P, NB, OCHUNK], f32)
            nc.sync.dma_start(
                out=nct,
                in_=n_in.ap()[:, sl].rearrange("(b p) f -> p b f", p=P))
            ot = opool.tile([P, NB, OCHUNK], f32)
            for b in range(NB):
                nc.vector.tensor_scalar(out=ot[:, b, :], in0=nct[:, b, :],
                                        scalar1=p[:, b:b + 1], scalar2=inv,
                                        op0=Alu.is_gt, op1=Alu.mult)
                nc.gpsimd.tensor_tensor(out=ot[:, b, :], in0=ot[:, b, :],
                                        in1=xt[:, b, sl], op=Alu.mult)
            nc.sync.dma_start(
                out=out_d.ap()[:, sl].rearrange("(b p) f -> p b f", p=P),
                in_=ot)

    nc.compile()
    return nc


_cache = {}


def get_program(epoch: int):
    if epoch not in _cache:
        factor4 = epoch + np.square(epoch * np.sin(epoch) * np.cos(epoch))
        inv3f4 = float(np.float32(1.0) / np.float32(3.0 * factor4))
        _cache[epoch] = build_program(inv3f4)
    return _cache[epoch]


def make_in_maps(x: np.ndarray, noise: np.ndarray):
    x = np.asarray(x, dtype=np.float32).reshape(B, F)
    noise = np.asarray(noise, dtype=np.float32).reshape(B, F)
    maps = []
    for i in range(NCORES):
        sl = slice(i * FS, (i + 1) * FS)
        maps.append({"x": np.ascontiguousarray(x[:, sl]),
                     "noise": np.ascontiguousarray(noise[:, sl])})
    return maps


def kernel(x, noise, epoch):
    from concourse.bass_utils import run_bass_kernel_spmd
    nc = get_program(int(epoch))
    in_maps = make_in_maps(x, noise)
    res = run_bass_kernel_spmd(nc, in_maps, core_ids=list(range(NCORES)))
    out = np.concatenate([res.results[i]["out"] for i in range(NCORES)], axis=1)
    return out.reshape(np.asarray(x).shape)


# revision 25
# speedup vs baseline: 1.2013x; 1.2013x over previous
"""DifferentialDropout TRN2 kernel.

Strategy (8 NeuronCores, column sharding):
  Each core owns a contiguous 8192-column slice of x/noise (full 256 rows).
  Phase A (local, per core):
    - G_k = X_k @ X_k^T  (256x256 partial Gram over the local columns) on PE,
      via per-128-column PE transpose + f32r matmuls accumulating in PSUM.
    - mean_k[i] = avg of x[i, local cols]  (Pool engine avg-pool)
    - hist_k[i] = OR over local cols of (1 << (round(x)+16))  (int32 bitmask
      of occupied integer bins; round == RNE via the fp32 magic-number trick,
      exactly matching jnp.round)
  AllGather of the 256x258 payload [G | mean | hist] (DRAM, 264KB).
  Phase B (every core, redundant 256-sized math):
    full G = sum_k G_k;  s8[i] = sum_k mean_k[i] (= 8*mean_i)
    mse*F = diag(G) - (2/B) rowsum(G) + totsum(G)/B^2   (scale-invariant in
      factor2 = mse_i / sum(mse))
    C = G - 1024 * s8_i * s8_j  (centered Gram);  d = 1/sqrt(diag C)
    factor1_i = (1/B) d_i * sum_j |C_ij| d_j
    row_unique = popcount(OR_k hist_k[i]);  total_unique = popcount(OR_all)
    p_i = 1 - (f1 + (1-f2) + (1-f3)) / (3*factor4)
  Phase C (local): out = (noise > p_i) * x * (1/(1-p[255])), streamed out.

All numerics fp32; Gram in float32r (full-rate fp32 matmul).
"""

import numpy as np
from contextlib import ExitStack

import concourse.bass as bass
import concourse.tile as tile
from concourse import mybir
from concourse.masks import make_identity
from concourse.bass_isa import ReduceOp

B = 256          # rows
F = 65536        # total columns
NCORES = 8
FS = F // NCORES  # 8192 columns per core
NB = 2            # row blocks of 128
P = 128
HCHUNK = 2048     # hist pass chunk width
OCHUNK = 1024     # output chunk width
MAGIC = 12582912.0  # 1.5 * 2**23 : fp32 RNE-to-integer magic
GRAM_F32R = True  # f32r matmul: 4x faster PE, reduced mantissa

f32 = mybir.dt.float32
f32r = mybir.dt.float32r
i32 = mybir.dt.int32
Alu = mybir.AluOpType
AX = mybir.AxisListType


def build_program(inv3f4: float):
    """Build the 8-core SPMD Bass program. inv3f4 = float32(1/(3*factor4))."""
    import concourse.bacc as bacc
    NCHUNK = FS // P          # transpose/matmul chunks
    SCOEF = 1.0 / F           # s_i*s_j scale for centering (power of 2)
    PAYW = B + 2              # payload cols: G | mean | hist
    nc = bacc.Bacc("TRN2", target_bir_lowering=False, debug=False,
                   num_devices=NCORES)
    x_in = nc.dram_tensor("x", [B, FS], f32, kind="ExternalInput")
    n_in = nc.dram_tensor("noise", [B, FS], f32, kind="ExternalInput")
    out_d = nc.dram_tensor("out", [B, FS], f32, kind="ExternalOutput")

    with tile.TileContext(nc) as tc, ExitStack() as ctx:
        big = ctx.enter_context(tc.tile_pool(name="big", bufs=1))
        xt = big.tile([P, NB, FS], f32)    # 64KB/part

        singles = ctx.enter_context(tc.tile_pool(name="singles", bufs=1))
        kpool = ctx.enter_context(tc.tile_pool(name="kp", bufs=2))
        fold = ctx.enter_context(tc.tile_pool(name="fold", bufs=2))
        opool = ctx.enter_context(tc.tile_pool(name="op", bufs=4))
        small = ctx.enter_context(tc.tile_pool(name="small", bufs=8))
        onep = ctx.enter_context(tc.tile_pool(name="onep", bufs=1))
        psg = ctx.enter_context(tc.tile_pool(name="psg", bufs=1, space="PSUM"))
        pst = ctx.enter_context(tc.tile_pool(name="pst", bufs=3, space="PSUM"))
        dram = ctx.enter_context(tc.tile_pool(name="dram", bufs=1, space="DRAM"))

        # ---- input DMA (x first; noise later, overlaps gram/collective) ----
        NXD = 8
        xw = FS // NXD
        for c in range(NXD):
            sl = slice(c * xw, (c + 1) * xw)
            nc.sync.dma_start(
                out=xt[:, :, sl],
                in_=x_in.ap()[:, sl].rearrange("(b p) f -> p b f", p=P))

        # ---- constants ----
        ident = singles.tile([P, P], f32)
        make_identity(nc, ident)
        one_i = singles.tile([P, 1], i32)
        nc.vector.memset(one_i, 1)

        # ---- Gram + row sums: rhs = [Xt_c | ones], psum col 256 = rowsums ----
        gdt = f32r if GRAM_F32R else f32
        c10 = singles.tile([P, 2], f32)     # [1.0, 0.0]: ones col + even-pad col
        nc.vector.memset(c10[:, 0:1], 1.0)
        nc.vector.memset(c10[:, 1:2], 0.0)
        xtcc = [singles.tile([P, B + 2], gdt, name=f"xtcc{i}") for i in range(3)]
        for i in range(3):
            nc.scalar.copy(out=xtcc[i][:, B:B + 2], in_=c10)
        gps = [psg.tile([P, B + 2], f32, name=f"gps{b}") for b in range(NB)]
        for c in range(NCHUNK):
            fsl = slice(c * P, (c + 1) * P)
            tp = pst.tile([P, B], f32)
            nc.tensor.transpose(tp[:, 0:P], xt[:, 0, fsl], ident)
            nc.tensor.transpose(tp[:, P:B], xt[:, 1, fsl], ident)
            xtc = xtcc[c % 3]
            nc.scalar.copy(out=xtc[:, 0:B], in_=tp)
            for b in range(NB):
                nc.tensor.matmul(gps[b],
                                 lhsT=xtc[:, b * P:(b + 1) * P],
                                 rhs=xtc,
                                 start=(c == 0), stop=(c == NCHUNK - 1))

        # ---- payload [G | rowsum | hist] ----
        payload = singles.tile([P, NB, PAYW], f32)
        for b in range(NB):
            nc.scalar.copy(out=payload[:, b, 0:B + 1], in_=gps[b][:, 0:B + 1])

        # hist bitmask, shift-free: k = RNE-round(x)+143 (fused magic round),
        # e = k * 2^23 (int mult on Pool) == f32 bits of 2.0^(round(x)+16),
        # m = int32(bitcast_f32(e)) == 1 << (round(x)+16), then OR-fold.
        c8m = singles.tile([P, 1], i32)
        nc.vector.memset(c8m, 1 << 23)
        hist = singles.tile([P, NB], i32)
        nc.vector.memset(hist, 0)
        for b in range(NB):
            for hc in range(FS // HCHUNK):
                sl = slice(hc * HCHUNK, (hc + 1) * HCHUNK)
                k = kpool.tile([P, HCHUNK], i32)
                nc.vector.tensor_scalar(out=k, in0=xt[:, b, sl],
                                        scalar1=MAGIC, scalar2=MAGIC - 143.0,
                                        op0=Alu.add, op1=Alu.subtract)
                nc.gpsimd.tensor_tensor(out=k, in0=k,
                                        in1=c8m.to_broadcast([P, HCHUNK]),
                                        op=Alu.mult)
                m = k
                nc.scalar.copy(out=m, in_=k.bitcast(f32))
                w = HCHUNK
                src = m
                while w > 1:
                    w //= 2
                    nxt = fold.tile([P, w], i32)
                    nc.vector.tensor_tensor(out=nxt, in0=src[:, 0:w],
                                            in1=src[:, w:2 * w],
                                            op=Alu.bitwise_or)
                    src = nxt
                nc.vector.tensor_tensor(out=hist[:, b:b + 1], in0=hist[:, b:b + 1],
                                        in1=src, op=Alu.bitwise_or)
        nc.vector.tensor_copy(out=payload[:, :, PAYW - 1:PAYW].bitcast(i32),
                              in_=hist.rearrange("p (b o) -> p b o", o=1))

        # ---- AllGather payload ----
        pay_d = dram.tile([P, NB, PAYW], f32)
        gath_d = dram.tile([NCORES, P, NB, PAYW], f32, addr_space="Shared")
        nc.sync.dma_start(out=pay_d, in_=payload)
        nc.gpsimd.collective_compute(
            "AllGather", Alu.bypass,
            replica_groups=[list(range(NCORES))],
            ins=[pay_d[:]], outs=[gath_d[:]])

        gsb = singles.tile([P, NCORES, NB, PAYW], f32)  # 16.5KB/part
        nc.sync.dma_start(out=gsb, in_=gath_d.rearrange("n p b c -> p n b c"))

        # ---- combine cores ----
        acc = singles.tile([P, NB, B + 1], f32)  # G (256) + s8 (1)
        nc.vector.tensor_reduce(
            out=acc, in_=gsb.rearrange("p n b c -> p b c n")[:, :, 0:B + 1, :],
            axis=AX.X, op=Alu.add)
        G = acc[:, :, 0:B]
        s8 = acc[:, :, B:B + 1]            # [P, NB, 1] = full row sum

        histg = gsb[:, :, :, PAYW - 1:PAYW].bitcast(i32)  # [P, 8, NB, 1]
        h4 = small.tile([P, 4, NB], i32)
        nc.vector.tensor_tensor(out=h4, in0=histg[:, 0:4, :, 0],
                                in1=histg[:, 4:8, :, 0], op=Alu.bitwise_or)
        h2 = small.tile([P, 2, NB], i32)
        nc.vector.tensor_tensor(out=h2, in0=h4[:, 0:2, :], in1=h4[:, 2:4, :],
                                op=Alu.bitwise_or)
        histf = small.tile([P, NB], i32)   # full-row bin mask
        nc.vector.tensor_tensor(out=histf, in0=h2[:, 0, :], in1=h2[:, 1, :],
                                op=Alu.bitwise_or)

        # ---- diag/rowsum of G ----
        eye2 = singles.tile([P, NB, B], f32)
        nc.vector.memset(eye2, 0.0)
        for b in range(NB):
            nc.vector.tensor_copy(out=eye2[:, b, b * P:(b + 1) * P], in_=ident)
        gd = small.tile([P, NB, B], f32)
        nc.vector.tensor_tensor(out=gd, in0=G, in1=eye2, op=Alu.mult)
        diagG = small.tile([P, NB], f32)
        nc.vector.tensor_reduce(out=diagG, in_=gd, axis=AX.X, op=Alu.add)
        rowsG = small.tile([P, NB], f32)
        nc.vector.tensor_reduce(out=rowsG, in_=G, axis=AX.X, op=Alu.add)
        # partition reductions via DMA round trip: pack [rowsum-total, diag-total]
        pr2 = small.tile([P, 2], f32)
        nc.vector.tensor_reduce(out=pr2[:, 0:1], in_=rowsG, axis=AX.X, op=Alu.add)
        nc.vector.tensor_reduce(out=pr2[:, 1:2], in_=diagG, axis=AX.X, op=Alu.add)
        pr_d = dram.tile([P, 2], f32)
        nc.sync.dma_start(out=pr_d, in_=pr2)
        prt = onep.tile([1, 2, P], f32)
        nc.sync.dma_start(out=prt,
                          in_=bass.AP(tensor=pr_d.tensor, offset=pr_d.offset,
                                      ap=[[0, 1], [1, 2], [2, P]]))
        pr1 = small.tile([1, 2], f32)     # [totG, sum diagG]
        nc.vector.tensor_reduce(out=pr1, in_=prt, axis=AX.X, op=Alu.add)

        # ---- centered Gram + factor1 ----
        # s8 broadcast to free dim via DRAM round trip
        srow_d = dram.tile([B], f32)
        nc.sync.dma_start(out=srow_d.rearrange("(b p) -> p b", p=P), in_=s8[:, :, 0])
        s8b = singles.tile([P, B], f32)
        nc.sync.dma_start(out=s8b, in_=bass.AP(tensor=srow_d.tensor,
                                               offset=srow_d.offset,
                                               ap=[[0, P]] + srow_d.ap))
        nB8 = small.tile([P, NB], f32)
        nc.vector.tensor_scalar(out=nB8, in0=s8, scalar1=-SCOEF, scalar2=None,
                                op0=Alu.mult)
        C = singles.tile([P, NB, B], f32)
        for b in range(NB):
            nc.vector.scalar_tensor_tensor(out=C[:, b, :], in0=s8b,
                                           scalar=nB8[:, b:b + 1],
                                           in1=G[:, b, :],
                                           op0=Alu.mult, op1=Alu.add)
        cd = small.tile([P, NB], f32)
        nc.vector.tensor_tensor(out=cd, in0=s8, in1=s8, op=Alu.mult)
        diagC = small.tile([P, NB], f32)
        nc.vector.scalar_tensor_tensor(out=diagC, in0=cd, scalar=-SCOEF,
                                       in1=diagG, op0=Alu.mult, op1=Alu.add)
        sdC = small.tile([P, NB], f32)
        nc.scalar.sqrt(out=sdC, in_=diagC)
        d = small.tile([P, NB], f32)
        nc.vector.reciprocal(out=d, in_=sdC)
        drow_d = dram.tile([B], f32)
        nc.sync.dma_start(out=drow_d.rearrange("(b p) -> p b", p=P), in_=d)
        db = singles.tile([P, B], f32)
        nc.sync.dma_start(out=db, in_=bass.AP(tensor=drow_d.tensor,
                                              offset=drow_d.offset,
                                              ap=[[0, P]] + drow_d.ap))
        v = small.tile([P, NB], f32)
        for b in range(NB):
            u = small.tile([P, B], f32)
            nc.vector.tensor_tensor(out=u, in0=C[:, b, :], in1=db, op=Alu.mult)
            nc.vector.tensor_reduce(out=v[:, b:b + 1], in_=u, axis=AX.X,
                                    op=Alu.add, apply_absolute_value=True)
        f1a = small.tile([P, NB], f32)
        nc.vector.tensor_tensor(out=f1a, in0=v, in1=d, op=Alu.mult)
        f1 = small.tile([P, NB], f32)
        nc.vector.tensor_scalar(out=f1, in0=f1a, scalar1=1.0 / B, scalar2=None,
                                op0=Alu.mult)

        # ---- unique counts ----
        def popcount(eng, t, w):
            """SWAR popcount of int32 tile t [P or 1, w] -> f32 tile."""
            def const(val):
                ct = small.tile([t.shape[0], 1], i32)
                nc.vector.memset(ct, val)
                return ct.to_broadcast([t.shape[0], w])
            def shr(src, amt):
                o = small.tile([t.shape[0], w], i32)
                eng.tensor_tensor(out=o, in0=src, in1=const(amt),
                                  op=Alu.logical_shift_right)
                return o
            def band(a, bmask):
                o = small.tile([t.shape[0], w], i32)
                eng.tensor_tensor(out=o, in0=a, in1=const(bmask), op=Alu.bitwise_and)
                return o
            def add(a, b2):
                o = small.tile([t.shape[0], w], i32)
                eng.tensor_tensor(out=o, in0=a, in1=b2, op=Alu.add)
                return o
            def sub(a, b2):
                o = small.tile([t.shape[0], w], i32)
                eng.tensor_tensor(out=o, in0=a, in1=b2, op=Alu.subtract)
                return o
            v1 = sub(t, band(shr(t, 1), 0x55555555))
            v2 = add(band(v1, 0x33333333), band(shr(v1, 2), 0x33333333))
            v3 = band(add(v2, shr(v2, 4)), 0x0F0F0F0F)
            v4 = add(v3, shr(v3, 8))
            v5 = band(add(v4, shr(v4, 16)), 0x3F)
            o = small.tile([t.shape[0], w], f32)
            eng.tensor_copy(out=o, in_=v5)
            return o

        ru = popcount(nc.vector, histf, NB)   # row unique counts, f32

        # total unique: OR the 2048 per-core-block masks from gathered DRAM
        thist = onep.tile([1, NCORES * NB * P], i32)
        nc.sync.dma_start(
            out=thist,
            in_=bass.AP(tensor=gath_d.tensor, offset=gath_d.offset + (PAYW - 1),
                        ap=[[0, 1], [PAYW * NB * P, NCORES], [PAYW * NB, P],
                            [PAYW, NB]]).bitcast(i32))
        w = NCORES * NB * P
        src = thist
        while w > 1:
            w //= 2
            nxt = fold.tile([1, w], i32)
            nc.vector.tensor_tensor(out=nxt, in0=src[:, 0:w], in1=src[:, w:2 * w],
                                    op=Alu.bitwise_or)
            src = nxt
        tu = popcount(nc.vector, src, 1)      # [1,1] f32

        # broadcast [totG, sum diagG, total_unique] to all partitions via DRAM
        b3 = small.tile([1, 3], f32)
        nc.vector.tensor_copy(out=b3[:, 0:2], in_=pr1)
        nc.vector.tensor_copy(out=b3[:, 2:3], in_=tu)
        b3_d = dram.tile([3], f32)
        nc.sync.dma_start(out=b3_d, in_=b3)
        bb = small.tile([P, 3], f32)
        nc.sync.dma_start(out=bb, in_=bass.AP(tensor=b3_d.tensor,
                                              offset=b3_d.offset,
                                              ap=[[0, P], [1, 3]]))
        totGb = bb[:, 0:1]

        # mse (x F; scale cancels in factor2). total = sum diagG - totG/B
        mse0 = small.tile([P, NB], f32)
        nc.vector.scalar_tensor_tensor(out=mse0, in0=rowsG, scalar=-2.0 / B,
                                       in1=diagG, op0=Alu.mult, op1=Alu.add)
        mse = small.tile([P, NB], f32)     # = F * mse_rows
        nc.vector.scalar_tensor_tensor(out=mse, in0=totGb.to_broadcast([P, NB]),
                                       scalar=1.0 / (B * B), in1=mse0,
                                       op0=Alu.mult, op1=Alu.add)
        tmseb = small.tile([P, 1], f32)
        nc.vector.scalar_tensor_tensor(out=tmseb, in0=bb[:, 0:1],
                                       scalar=-1.0 / B, in1=bb[:, 1:2],
                                       op0=Alu.mult, op1=Alu.add)
        rtmse = small.tile([P, 1], f32)
        nc.vector.reciprocal(out=rtmse, in_=tmseb)
        f2 = small.tile([P, NB], f32)
        nc.vector.tensor_scalar(out=f2, in0=mse, scalar1=rtmse, scalar2=None,
                                op0=Alu.mult)

        rtu = small.tile([P, 1], f32)
        nc.vector.reciprocal(out=rtu, in_=bb[:, 2:3])
        f3 = small.tile([P, NB], f32)
        nc.vector.tensor_scalar(out=f3, in0=ru, scalar1=rtu, scalar2=None,
                                op0=Alu.mult)

        # ---- p = 1 - (f1 + (1-f2) + (1-f3)) * inv3f4 ----
        t1 = small.tile([P, NB], f32)
        nc.vector.tensor_scalar(out=t1, in0=f2, scalar1=-1.0, scalar2=1.0,
                                op0=Alu.mult, op1=Alu.add)
        t2 = small.tile([P, NB], f32)
        nc.vector.tensor_tensor(out=t2, in0=f1, in1=t1, op=Alu.add)
        t3 = small.tile([P, NB], f32)
        nc.vector.tensor_scalar(out=t3, in0=f3, scalar1=-1.0, scalar2=1.0,
                                op0=Alu.mult, op1=Alu.add)
        t4 = small.tile([P, NB], f32)
        nc.vector.tensor_tensor(out=t4, in0=t2, in1=t3, op=Alu.add)
        p = small.tile([P, NB], f32)
        nc.vector.tensor_scalar(out=p, in0=t4, scalar1=-inv3f4, scalar2=1.0,
                                op0=Alu.mult, op1=Alu.add)

        # 1/(1 - p_last), p_last = p[row 255] = p[part 127, block 1]
        pl_d = dram.tile([1], f32)
        nc.sync.dma_start(out=pl_d, in_=p[127:128, 1:2])
        plastb = small.tile([P, 1], f32)
        nc.sync.dma_start(out=plastb, in_=bass.AP(tensor=pl_d.tensor,
                                                  offset=pl_d.offset,
                                                  ap=[[0, P], [1, 1]]))
        om = small.tile([P, 1], f32)
        nc.vector.tensor_scalar(out=om, in0=plastb, scalar1=-1.0, scalar2=1.0,
                                op0=Alu.mult, op1=Alu.add)
        inv = small.tile([P, 1], f32)
        nc.vector.reciprocal(out=inv, in_=om)

        # ---- masked scaled output ----
        # Pre-scale x by 1/(1-p_last) in place on ACT (chunked), then per
        # chunk a single fused (noise > p) * xs op, DVE block 0 / Pool
        # block 1. Noise streams on the ACT DGE queue so its DMAs are not
        # blocked behind the collective wait on the SP sequencer.
        for oc in range(FS // OCHUNK):
            sl = slice(oc * OCHUNK, (oc + 1) * OCHUNK)
            for b in range(NB):
                nc.scalar.activation(out=xt[:, b, sl], in_=xt[:, b, sl],
                                     func=mybir.ActivationFunctionType.Copy,
                                     scale=inv)
        for oc in range(FS // OCHUNK):
            sl = slice(oc * OCHUNK, (oc + 1) * OCHUNK)
            nct = opool.tile([P, NB, OCHUNK], f32)
            nc.scalar.dma_start(
                out=nct,
                in_=n_in.ap()[:, sl].rearrange("(b p) f -> p b f", p=P))
            for b in range(NB):
                nc.vector.scalar_tensor_tensor(
                    out=nct[:, b, :], in0=nct[:, b, :], scalar=p[:, b:b + 1],
                    in1=xt[:, b, sl], op0=Alu.is_gt, op1=Alu.mult)
            nc.sync.dma_start(
                out=out_d.ap()[:, sl].rearrange("(b p) f -> p b f", p=P),
                in_=nct)

    nc.compile()
    return nc


_cache = {}


def get_program(epoch: int):
    if epoch not in _cache:
        factor4 = epoch + np.square(epoch * np.sin(epoch) * np.cos(epoch))
        inv3f4 = float(np.float32(1.0) / np.float32(3.0 * factor4))
        _cache[epoch] = build_program(inv3f4)
    return _cache[epoch]


def make_in_maps(x: np.ndarray, noise: np.ndarray):
    x = np.asarray(x, dtype=np.float32).reshape(B, F)
    noise = np.asarray(noise, dtype=np.float32).reshape(B, F)
    maps = []
    for i in range(NCORES):
        sl = slice(i * FS, (i + 1) * FS)
        maps.append({"x": np.ascontiguousarray(x[:, sl]),
                     "noise": np.ascontiguousarray(noise[:, sl])})
    return maps


def kernel(x, noise, epoch):
    from concourse.bass_utils import run_bass_kernel_spmd
    nc = get_program(int(epoch))
    in_maps = make_in_maps(x, noise)
    res = run_bass_kernel_spmd(nc, in_maps, core_ids=list(range(NCORES)))
    out = np.concatenate([res.results[i]["out"] for i in range(NCORES)], axis=1)
    return out.reshape(np.asarray(x).shape)
